# revision 13
# baseline (speedup 1.0000x reference)
"""BusNode kernel for Trainium2 (8 NeuronCores, data-parallel over tokens).

Self-contained: hardcodes shapes from the problem spec.
  token_state [4,2048,1024] f32, bus_symbols [8,4,2048,256] f32,
  bus_indices [8,4,2048] (unused by reference), bus_outputs [8,4,2048,1024] f32,
  bus_mask [8] (unused). Outputs: node_output, quantized, indices, keep_mask.

Numerics strategy (validated against the fixed key(0) inputs):
 - relevance + argmax over T: DVE fp32 (exact vs reference, min margin 6.3e-4)
 - VQ index path: raw_symbol = z @ (Wread@Wsym fused on host, fp32) via fp32 PE
   matmuls (4 cyc/row); 0 argmin flips, min margin 2.7e-4 >> fp32 noise ~1e-5
 - continuous path (z_read, Wc1, Wc2): float32r (13-bit mantissa, 1 cyc/row),
   ~2e-4 relative error on node_output
"""
import sys

sys.path.insert(0, "/opt/trn_rl_repo")

import numpy as np

T, B, S = 8, 4, 2048
LAT, SYM, NCODE = 1024, 256, 512
NCORES = 8
N = B * S               # 8192 tokens
NT = N // NCORES        # 1024 tokens per core
P = 128
NTILES = NT // P        # 8 tiles/core
HALF = 512              # tokens per half
HT = 2                  # halves
TPH = 4                 # tiles per half

_CACHE = {}


def _round_f32r(x: np.ndarray) -> np.ndarray:
    """Round fp32 to 13-bit mantissa (float32r grid), RN-ish."""
    b = np.ascontiguousarray(x, dtype=np.float32).view(np.uint32)
    r = (b + 0x200 + ((b >> 10) & 1)) & np.uint32(0xFFFFFC00)
    return r.view(np.float32)


def _build():
    from concourse import bacc, tile
    import concourse.bass as bass
    import concourse.mybir as mybir
    from concourse.masks import make_identity

    f32 = mybir.dt.float32
    f32r = mybir.dt.float32r
    u32 = mybir.dt.uint32
    Alu = mybir.AluOpType

    nc = bacc.Bacc("TRN2", target_bir_lowering=False)

    ts_d = nc.dram_tensor("ts", [NT, LAT], f32, kind="ExternalInput")
    syms_d = nc.dram_tensor("syms", [T, NT, SYM], f32, kind="ExternalInput")
    outs_d = nc.dram_tensor("outs", [T, NT, LAT], f32, kind="ExternalInput")
    wq_d = nc.dram_tensor("wq", [1, SYM], f32, kind="ExternalInput")
    wread_d = nc.dram_tensor("wread", [2 * LAT, LAT], f32, kind="ExternalInput")
    wrs_d = nc.dram_tensor("wrs", [2 * LAT, SYM], f32, kind="ExternalInput")
    wc1_d = nc.dram_tensor("wc1", [LAT + SYM, LAT], f32, kind="ExternalInput")
    wc2_d = nc.dram_tensor("wc2", [LAT, LAT], f32, kind="ExternalInput")
    cb_d = nc.dram_tensor("cb", [NCODE, SYM], f32, kind="ExternalInput")

    node_d = nc.dram_tensor("node", [NT, LAT], f32, kind="ExternalOutput")
    quant_d = nc.dram_tensor("quant", [NT, SYM], f32, kind="ExternalOutput")
    vqidx_d = nc.dram_tensor("vqidx", [NTILES, 4, 32], u32, kind="ExternalOutput")
    tcnt_d = nc.dram_tensor("tcnt", [1, 8], f32, kind="ExternalOutput")

    KI = 2 * LAT // P   # 16 contraction chunks of z
    KO_R = LAT // P     # 8 output chunks of z_read
    KC1 = (LAT + SYM) // P  # 10 contraction chunks for Wc1

    with tile.TileContext(nc) as tc:
        with (
            tc.tile_pool(name="const", bufs=1) as cpool,
            tc.tile_pool(name="big", bufs=1) as bpool,
            tc.tile_pool(name="stream", bufs=3) as spool,
            tc.tile_pool(name="work", bufs=2) as wpool,
            tc.tile_pool(name="out", bufs=2) as opool,
            tc.tile_pool(name="ps_wr", bufs=2, space="PSUM") as ps_wr,
            tc.tile_pool(name="ps_mid", bufs=2, space="PSUM") as ps_mid,
            tc.tile_pool(name="ps_wc2", bufs=2, space="PSUM") as ps_wc2,
            tc.tile_pool(name="ps_sm", bufs=2, space="PSUM") as ps_sm,
        ):
            # ---------- setup ----------
            ident = cpool.tile([P, P], f32, tag="ident")
            make_identity(nc, ident[:])

            wq1 = cpool.tile([1, SYM], f32, tag="wq1")
            nc.sync.dma_start(wq1[:], wq_d[:])
            wqb = cpool.tile([P, SYM], f32, tag="wqb")
            nc.gpsimd.partition_broadcast(wqb[:], wq1[:])

            ones_col = cpool.tile([P, 1], f32, tag="ones_col")
            nc.vector.memset(ones_col[:], 1.0)
            ones_row = cpool.tile([1, P], f32, tag="ones_row")
            nc.vector.memset(ones_row[:], 1.0)

            # codebook -> cbT [sym, code] and c2b = broadcast ||c||^2
            cbT = cpool.tile([P, SYM // P, NCODE], f32, tag="cbT")
            c2b = cpool.tile([P, NCODE], f32, tag="c2b")
            cbsb = wpool.tile([P, NCODE // P, SYM], f32, tag="st", name="cbsb")
            nc.sync.dma_start(
                cbsb[:], cb_d.ap().rearrange("(c p) s -> p c s", p=P)
            )
            for c in range(NCODE // P):
                for sj in range(SYM // P):
                    tp = ps_sm.tile([P, P], f32, tag="sm")
                    nc.tensor.transpose(
                        tp[:], cbsb[:, c, sj * P : (sj + 1) * P], ident[:]
                    )
                    nc.vector.tensor_copy(
                        cbT[:, sj, c * P : (c + 1) * P], tp[:]
                    )
            cbTsq = wpool.tile([P, SYM // P, NCODE], f32, tag="st", name="cbTsq")
            nc.vector.tensor_mul(cbTsq[:], cbT[:], cbT[:])
            c2ps = ps_sm.tile([1, NCODE], f32, tag="sm")
            for sj in range(SYM // P):
                nc.tensor.matmul(
                    c2ps[:], ones_col[:], cbTsq[:, sj, :],
                    start=(sj == 0), stop=(sj == SYM // P - 1),
                )
            c2row = cpool.tile([1, NCODE], f32, tag="c2row")
            nc.vector.tensor_copy(c2row[:], c2ps[:])
            c2b_ps = ps_sm.tile([P, NCODE], f32, tag="sm")
            nc.tensor.matmul(
                c2b_ps[:], ones_row[:], c2row[:], start=True, stop=True
            )
            nc.vector.tensor_copy(c2b[:], c2b_ps[:])

            cnt_sb = cpool.tile([1, 8], f32, tag="cnt_sb")
            nc.vector.memset(cnt_sb[:], 0.0)

            # ---------- persistent per-half activations ----------
            zT = bpool.tile([P, KI, HALF], f32, tag="zT")
            zTr = bpool.tile([P, KI, HALF], f32r, tag="zTr")
            zread = bpool.tile([P, KO_R, HALF], f32r, tag="zread")
            rsT = bpool.tile([P, SYM // P, HALF], f32, tag="rsT")
            qT = bpool.tile([P, SYM // P, HALF], f32r, tag="qT")
            hT = bpool.tile([P, KO_R, HALF], f32r, tag="hT")
            ts_half = bpool.tile([P, TPH, LAT], f32, tag="ts_half")

            outs_flat = outs_d.ap().rearrange("t n l -> (t n) l")
            syms_tok = syms_d.ap().transpose([1, 0, 2])  # [NT, T, SYM]

            for h in range(HT):
                # ---------- stage A: relevance/argmax/gather/transpose ----------
                for i in range(TPH):
                    g = h * TPH + i        # global tile id
                    n0 = g * P             # first token of tile
                    st = wpool.tile([P, T, SYM], f32, tag="st")
                    nc.sync.dma_start(st[:], syms_tok[n0 : n0 + P])

                    rel = wpool.tile([P, 8], f32, tag="rel")
                    scr = wpool.tile([P, SYM], f32, tag="scr")
                    for t in range(T):
                        nc.vector.scalar_tensor_tensor(
                            out=scr[:], in0=st[:, t], scalar=1.0, in1=wqb[:],
                            op0=Alu.mult, op1=Alu.mult,
                            accum_out=rel[:, t : t + 1],
                        )
                    mxr = wpool.tile([P, 8], f32, tag="mxr")
                    mir = wpool.tile([P, 8], u32, tag="mir")
                    nc.vector.max(mxr[:], rel[:])
                    nc.vector.max_index(mir[:], mxr[:], rel[:])

                    oh = wpool.tile([P, 8], f32, tag="oh")
                    nc.vector.tensor_scalar(
                        oh[:], rel[:], mxr[:, :1], None, Alu.is_equal
                    )
                    cps = ps_sm.tile([1, 8], f32, tag="sm")
                    nc.tensor.matmul(cps[:], ones_col[:], oh[:], start=True, stop=True)
                    nc.vector.tensor_add(cnt_sb[:], cnt_sb[:], cps[:])

                    io = wpool.tile([P, 1], u32, tag="io")
                    nc.gpsimd.iota(
                        io[:], pattern=[[0, 1]], base=n0, channel_multiplier=1
                    )
                    off = wpool.tile([P, 1], u32, tag="off")
                    nc.vector.scalar_tensor_tensor(
                        out=off[:], in0=mir[:, :1], scalar=float(NT), in1=io[:],
                        op0=Alu.mult, op1=Alu.add,
                    )

                    ctx = wpool.tile([P, LAT], f32, tag="ctx")
                    nc.gpsimd.indirect_dma_start(
                        out=ctx[:], out_offset=None, in_=outs_flat,
                        in_offset=bass.IndirectOffsetOnAxis(ap=off[:], axis=0),
                    )
                    nc.sync.dma_start(
                        ts_half[:, i, :], ts_d[n0 : n0 + P, :]
                    )
                    # transpose ts block f -> zT[:, f, i*128:...], ctx -> zT[:, 8+f, ...]
                    for f in range(LAT // P):
                        tp = ps_sm.tile([P, P], f32, tag="sm")
                        nc.tensor.transpose(
                            tp[:], ts_half[:, i, f * P : (f + 1) * P], ident[:]
                        )
                        nc.vector.tensor_copy(zT[:, f, i * P : (i + 1) * P], tp[:])
                        nc.vector.tensor_copy(zTr[:, f, i * P : (i + 1) * P], tp[:])
                        tp2 = ps_sm.tile([P, P], f32, tag="sm")
                        nc.tensor.transpose(
                            tp2[:], ctx[:, f * P : (f + 1) * P], ident[:]
                        )
                        nc.vector.tensor_copy(
                            zT[:, 8 + f, i * P : (i + 1) * P], tp2[:]
                        )
                        nc.vector.tensor_copy(
                            zTr[:, 8 + f, i * P : (i + 1) * P], tp2[:]
                        )

                # ---------- stage B: z_read = z @ Wread (f32r) ----------
                for s in range(4):          # ko sweeps of 2
                    pk = [ps_wr.tile([P, HALF], f32, tag="wr", name=f"pk{h}_{s}_{j}") for j in range(2)]
                    for ki in range(KI):
                        wchf = spool.tile([P, 2 * P], f32, tag="wread_chf")
                        nc.sync.dma_start(
                            wchf[:],
                            wread_d[ki * P : (ki + 1) * P,
                                    s * 2 * P : (s + 1) * 2 * P],
                        )
                        wch = spool.tile([P, 2 * P], f32r, tag="wread_ch")
                        nc.vector.tensor_copy(wch[:], wchf[:])
                        for j in range(2):
                            nc.tensor.matmul(
                                pk[j][:], wch[:, j * P : (j + 1) * P],
                                zTr[:, ki, :],
                                start=(ki == 0), stop=(ki == KI - 1),
                            )
                    for j in range(2):
                        nc.vector.tensor_copy(zread[:, s * 2 + j, :], pk[j][:])

                # ---------- stage C: raw_symbol = z @ Wrs (fp32) ----------
                prs = [ps_mid.tile([P, HALF], f32, tag="mid", name=f"prs{h}_{j}") for j in range(2)]
                for ki in range(KI):
                    wch = spool.tile([P, SYM], f32, tag="wrs_ch")
                    nc.sync.dma_start(
                        wch[:], wrs_d[ki * P : (ki + 1) * P, :]
                    )
                    for j in range(2):
                        nc.tensor.matmul(
                            prs[j][:], wch[:, j * P : (j + 1) * P], zT[:, ki, :],
                            start=(ki == 0), stop=(ki == KI - 1),
                        )
                for j in range(2):
                    nc.vector.tensor_copy(rsT[:, j, :], prs[j][:])

                # ---------- stage D: VQ argmin + quantized ----------
                for i in range(TPH):
                    g = h * TPH + i
                    n0 = g * P
                    pvq = ps_sm.tile([P, NCODE], f32, tag="sm")
                    for j in range(SYM // P):
                        nc.tensor.matmul(
                            pvq[:], rsT[:, j, i * P : (i + 1) * P], cbT[:, j, :],
                            start=(j == 0), stop=(j == SYM // P - 1),
                        )
                    sc = wpool.tile([P, NCODE], f32, tag="svq")
                    nc.vector.scalar_tensor_tensor(
                        out=sc[:], in0=pvq[:], scalar=2.0, in1=c2b[:],
                        op0=Alu.mult, op1=Alu.subtract,
                    )
                    mxv = wpool.tile([P, 8], f32, tag="mxv")
                    miv = wpool.tile([P, 8], u32, tag="miv")
                    nc.vector.max(mxv[:], sc[:])
                    nc.vector.max_index(miv[:], mxv[:], sc[:])

                    qt = wpool.tile([P, SYM], f32, tag="qt")
                    nc.gpsimd.indirect_dma_start(
                        out=qt[:], out_offset=None, in_=cb_d[:],
                        in_offset=bass.IndirectOffsetOnAxis(ap=miv[:, :1], axis=0),
                    )
                    nc.sync.dma_start(quant_d[n0 : n0 + P, :], qt[:])
                    for j in range(SYM // P):
                        tp = ps_sm.tile([P, P], f32, tag="sm")
                        nc.tensor.transpose(
                            tp[:], qt[:, j * P : (j + 1) * P], ident[:]
                        )
                        nc.vector.tensor_copy(qT[:, j, i * P : (i + 1) * P], tp[:])
                    wide = wpool.tile([P, 32], u32, tag="wide")
                    nc.vector.memset(wide[:], 0)
                    nc.vector.tensor_copy(wide[:, :1], miv[:, :1])
                    tr = wpool.tile([P, 32], u32, tag="tr")
                    nc.vector.transpose(tr[:], wide[:])
                    nc.sync.dma_start(
                        vqidx_d[g],
                        tr[:].rearrange("(b i) f -> b i f", i=32)[:, 0, :],
                    )

                # ---------- stage E: h = relu([z_read, q] @ Wc1) (f32r) ----------
                for s in range(4):          # ko sweeps of 2
                    pc = [ps_mid.tile([P, HALF], f32, tag="mid", name=f"pc{h}_{s}_{j}") for j in range(2)]
                    for ki in range(KC1):
                        wchf = spool.tile([P, 2 * P], f32, tag="wc1_chf")
                        nc.sync.dma_start(
                            wchf[:],
                            wc1_d[ki * P : (ki + 1) * P,
                                  s * 2 * P : (s + 1) * 2 * P],
                        )
                        wch = spool.tile([P, 2 * P], f32r, tag="wc1_ch")
                        nc.vector.tensor_copy(wch[:], wchf[:])
                        rhs = zread[:, ki, :] if ki < KO_R else qT[:, ki - KO_R, :]
                        for j in range(2):
                            nc.tensor.matmul(
                                pc[j][:], wch[:, j * P : (j + 1) * P], rhs,
                                start=(ki == 0), stop=(ki == KC1 - 1),
                            )
                    for j in range(2):
                        nc.vector.tensor_relu(hT[:, s * 2 + j, :], pc[j][:])

                # ---------- stage F: node = h @ Wc2 + ts (f32r + residual) ----------
                for ko in range(2):         # output col halves of 512
                    wc2s = bpool.tile([P, KO_R, HALF], f32r, tag="wc2_ch")
                    for ki in range(KO_R):
                        wchf = spool.tile([P, HALF], f32, tag="wc2_chf")
                        nc.sync.dma_start(
                            wchf[:],
                            wc2_d[ki * P : (ki + 1) * P,
                                  ko * HALF : (ko + 1) * HALF],
                        )
                        nc.vector.tensor_copy(wc2s[:, ki, :], wchf[:])
                    for i in range(TPH):
                        g = h * TPH + i
                        n0 = g * P
                        pn = ps_wc2.tile([P, HALF], f32, tag="wc2")
                        for ki in range(KO_R):
                            nc.tensor.matmul(
                                pn[:], hT[:, ki, i * P : (i + 1) * P],
                                wc2s[:, ki, :],
                                start=(ki == 0), stop=(ki == KO_R - 1),
                            )
                        nod = opool.tile([P, HALF], f32, tag="nod")
                        nc.vector.tensor_add(
                            nod[:], pn[:],
                            ts_half[:, i, ko * HALF : (ko + 1) * HALF],
                        )
                        nc.sync.dma_start(
                            node_d[n0 : n0 + P, ko * HALF : (ko + 1) * HALF],
                            nod[:],
                        )

            nc.sync.dma_start(tcnt_d[:], cnt_sb[:])

    nc.compile()
    return nc


def _get_nc():
    if "nc" not in _CACHE:
        _CACHE["nc"] = _build()
    return _CACHE["nc"]


def kernel(token_state, bus_symbols, bus_indices, bus_outputs, bus_mask,
           Wq, bq, Wread, bread, Wsym, bsym, Wc1, bc1, Wc2, bc2, codebook):
    from concourse.bass_utils import run_bass_kernel_spmd

    f32 = np.float32
    ts = np.ascontiguousarray(np.asarray(token_state, f32).reshape(N, LAT))
    syms = np.asarray(bus_symbols, f32).reshape(T, N, SYM)
    outs = np.asarray(bus_outputs, f32).reshape(T, N, LAT)
    Wq = np.asarray(Wq, f32)
    Wread_np = np.asarray(Wread, f32)
    Wsym_np = np.asarray(Wsym, f32)
    Wc1_np = np.asarray(Wc1, f32)
    Wc2_np = np.asarray(Wc2, f32)
    cb = np.ascontiguousarray(np.asarray(codebook, f32))

    for b in (bq, bread, bsym, bc1, bc2):
        assert not np.any(np.asarray(b)), "kernel assumes zero biases"

    wrs = np.ascontiguousarray(Wread_np @ Wsym_np)          # host-fused, fp32
    wq_row = np.ascontiguousarray(Wq.reshape(1, SYM))

    in_maps = []
    for c in range(NCORES):
        sl = slice(c * NT, (c + 1) * NT)
        in_maps.append({
            "ts": np.ascontiguousarray(ts[sl]),
            "syms": np.ascontiguousarray(syms[:, sl]),
            "outs": np.ascontiguousarray(outs[:, sl]),
            "wq": wq_row,
            "wread": np.ascontiguousarray(Wread_np),
            "wrs": wrs,
            "wc1": np.ascontiguousarray(Wc1_np),
            "wc2": np.ascontiguousarray(Wc2_np),
            "cb": cb,
        })

    nc = _get_nc()
    res = run_bass_kernel_spmd(nc, in_maps, core_ids=list(range(NCORES)))

    node = np.concatenate([res.results[c]["node"] for c in range(NCORES)], axis=0)
    quant = np.concatenate([res.results[c]["quant"] for c in range(NCORES)], axis=0)
    idx = np.concatenate(
        [res.results[c]["vqidx"].reshape(NT) for c in range(NCORES)], axis=0
    ).astype(np.int32)
    counts = np.zeros(8, np.float64)
    for c in range(NCORES):
        counts += res.results[c]["tcnt"].reshape(8).astype(np.float64)
    keep_mask = counts == 0

    return (
        node.reshape(B, S, LAT),
        quant.reshape(B, S, SYM),
        idx.reshape(B, S),
        keep_mask,
    )


# revision 20
# speedup vs baseline: 70.3080x; 70.3080x over previous
"""BusNode kernel for Trainium2 (8 NeuronCores, data-parallel over tokens).

Self-contained: hardcodes shapes from the problem spec.
  token_state [4,2048,1024] f32, bus_symbols [8,4,2048,256] f32,
  bus_indices [8,4,2048] (unused by reference), bus_outputs [8,4,2048,1024] f32,
  bus_mask [8] (unused). Outputs: node_output, quantized, indices, keep_mask.

Numerics strategy (validated against the fixed key(0) inputs):
 - relevance + argmax over T: DVE fp32 (exact vs reference, min margin 6.3e-4)
 - VQ index path: raw_symbol = z @ (Wread@Wsym fused on host, fp32) via fp32 PE
   matmuls (4 cyc/row); 0 argmin flips, min margin 2.7e-4 >> fp32 noise ~1e-5
 - continuous path (z_read, Wc1, Wc2): float32r (13-bit mantissa, 1 cyc/row),
   ~2e-4 relative error on node_output
"""
import sys

sys.path.insert(0, "/opt/trn_rl_repo")

import numpy as np

T, B, S = 8, 4, 2048
LAT, SYM, NCODE = 1024, 256, 512
NCORES = 8
N = B * S               # 8192 tokens
NT = N // NCORES        # 1024 tokens per core
P = 128
NTILES = NT // P        # 8 tiles/core
HALF = 512              # tokens per half
HT = 2                  # halves
TPH = 4                 # tiles per half

_CACHE = {}


def _round_f32r(x: np.ndarray) -> np.ndarray:
    """Round fp32 to 13-bit mantissa (float32r grid), RN-ish."""
    b = np.ascontiguousarray(x, dtype=np.float32).view(np.uint32)
    r = (b + 0x200 + ((b >> 10) & 1)) & np.uint32(0xFFFFFC00)
    return r.view(np.float32)


def _build():
    from concourse import bacc, tile
    import concourse.bass as bass
    import concourse.mybir as mybir
    from concourse.masks import make_identity

    f32 = mybir.dt.float32
    f32r = mybir.dt.float32r
    u32 = mybir.dt.uint32
    Alu = mybir.AluOpType

    nc = bacc.Bacc("TRN2", target_bir_lowering=False)

    ts_d = nc.dram_tensor("ts", [NT, LAT], f32, kind="ExternalInput")
    syms_d = nc.dram_tensor("syms", [T, NT, SYM], f32, kind="ExternalInput")
    outs_d = nc.dram_tensor("outs", [T, NT, LAT], f32, kind="ExternalInput")
    wq_d = nc.dram_tensor("wq", [1, SYM], f32, kind="ExternalInput")
    wread_d = nc.dram_tensor("wread", [2 * LAT, LAT], f32, kind="ExternalInput")
    wrs_d = nc.dram_tensor("wrs", [2 * LAT, SYM], f32, kind="ExternalInput")
    wc1_d = nc.dram_tensor("wc1", [LAT + SYM, LAT], f32, kind="ExternalInput")
    wc2_d = nc.dram_tensor("wc2", [LAT, LAT], f32, kind="ExternalInput")
    cb_d = nc.dram_tensor("cb", [NCODE, SYM], f32, kind="ExternalInput")

    node_d = nc.dram_tensor("node", [NT, LAT], f32, kind="ExternalOutput")
    quant_d = nc.dram_tensor("quant", [NT, SYM], f32, kind="ExternalOutput")
    vqidx_d = nc.dram_tensor("vqidx", [NTILES, 4, 32], u32, kind="ExternalOutput")
    tcnt_d = nc.dram_tensor("tcnt", [1, 8], f32, kind="ExternalOutput")

    KI = 2 * LAT // P   # 16 contraction chunks of z
    KO_R = LAT // P     # 8 output chunks of z_read
    KC1 = (LAT + SYM) // P  # 10 contraction chunks for Wc1

    with tile.TileContext(nc) as tc:
        with (
            tc.tile_pool(name="const", bufs=1) as cpool,
            tc.tile_pool(name="big", bufs=1) as bpool,
            tc.tile_pool(name="stream", bufs=3) as spool,
            tc.tile_pool(name="work", bufs=2) as wpool,
            tc.tile_pool(name="out", bufs=2) as opool,
            tc.tile_pool(name="ps_wr", bufs=2, space="PSUM") as ps_wr,
            tc.tile_pool(name="ps_mid", bufs=2, space="PSUM") as ps_mid,
            tc.tile_pool(name="ps_wc2", bufs=2, space="PSUM") as ps_wc2,
            tc.tile_pool(name="ps_sm", bufs=2, space="PSUM") as ps_sm,
        ):
            # ---------- setup ----------
            ident = cpool.tile([P, P], f32, tag="ident")
            make_identity(nc, ident[:])

            wq1 = cpool.tile([1, SYM], f32, tag="wq1")
            nc.sync.dma_start(wq1[:], wq_d[:])
            wqb = cpool.tile([P, SYM], f32, tag="wqb")
            nc.gpsimd.partition_broadcast(wqb[:], wq1[:])

            ones_col = cpool.tile([P, 1], f32, tag="ones_col")
            nc.vector.memset(ones_col[:], 1.0)
            ones_row = cpool.tile([1, P], f32, tag="ones_row")
            nc.vector.memset(ones_row[:], 1.0)

            # codebook -> cbT [sym, code] and c2b = broadcast ||c||^2
            cbT = cpool.tile([P, SYM // P, NCODE], f32, tag="cbT")
            c2b = cpool.tile([P, NCODE], f32, tag="c2b")
            cbsb = wpool.tile([P, NCODE // P, SYM], f32, tag="st", name="cbsb")
            nc.sync.dma_start(
                cbsb[:], cb_d.ap().rearrange("(c p) s -> p c s", p=P)
            )
            for c in range(NCODE // P):
                for sj in range(SYM // P):
                    tp = ps_sm.tile([P, P], f32, tag="sm")
                    nc.tensor.transpose(
                        tp[:], cbsb[:, c, sj * P : (sj + 1) * P], ident[:]
                    )
                    nc.vector.tensor_copy(
                        cbT[:, sj, c * P : (c + 1) * P], tp[:]
                    )
            cbTsq = wpool.tile([P, SYM // P, NCODE], f32, tag="st", name="cbTsq")
            nc.vector.tensor_mul(cbTsq[:], cbT[:], cbT[:])
            c2ps = ps_sm.tile([1, NCODE], f32, tag="sm")
            for sj in range(SYM // P):
                nc.tensor.matmul(
                    c2ps[:], ones_col[:], cbTsq[:, sj, :],
                    start=(sj == 0), stop=(sj == SYM // P - 1),
                )
            c2row = cpool.tile([1, NCODE], f32, tag="c2row")
            nc.vector.tensor_copy(c2row[:], c2ps[:])
            c2b_ps = ps_sm.tile([P, NCODE], f32, tag="sm")
            nc.tensor.matmul(
                c2b_ps[:], ones_row[:], c2row[:], start=True, stop=True
            )
            nc.vector.tensor_copy(c2b[:], c2b_ps[:])

            cnt_sb = cpool.tile([1, 8], f32, tag="cnt_sb")
            nc.vector.memset(cnt_sb[:], 0.0)

            # ---------- persistent per-half activations ----------
            zT = bpool.tile([P, KI, HALF], f32, tag="zT")
            zTr = bpool.tile([P, KI, HALF], f32r, tag="zTr")
            zread = bpool.tile([P, KO_R, HALF], f32r, tag="zread")
            rsT = bpool.tile([P, SYM // P, HALF], f32, tag="rsT")
            qT = bpool.tile([P, SYM // P, HALF], f32r, tag="qT")
            hT = bpool.tile([P, KO_R, HALF], f32r, tag="hT")
            ts_half = bpool.tile([P, TPH, LAT], f32, tag="ts_half")

            outs_flat = outs_d.ap().rearrange("t n l -> (t n) l")
            syms_tok = syms_d.ap().transpose([1, 0, 2])  # [NT, T, SYM]

            for h in range(HT):
                # ---------- stage A: relevance/argmax/gather/transpose ----------
                _sA = nc.named_scope(f"A{h}"); _sA.__enter__()
                for i in range(TPH):
                    g = h * TPH + i        # global tile id
                    n0 = g * P             # first token of tile
                    rel = wpool.tile([P, 8], f32, tag="rel")
                    scr = wpool.tile([P, NCODE], f32, tag="svq", name=f"scr{g}")
                    for tb in range(2):
                        st = wpool.tile([P, T // 2, SYM], f32, tag="st")
                        nc.scalar.dma_start(
                            st[:], syms_tok[n0 : n0 + P, tb * 4 : (tb + 1) * 4]
                        )
                        for t in range(T // 2):
                            nc.vector.scalar_tensor_tensor(
                                out=scr[:, :SYM], in0=st[:, t], scalar=1.0,
                                in1=wqb[:], op0=Alu.mult, op1=Alu.mult,
                                accum_out=rel[:, tb * 4 + t : tb * 4 + t + 1],
                            )
                    mxr = wpool.tile([P, 8], f32, tag="mxr")
                    mir = wpool.tile([P, 8], u32, tag="mir")
                    nc.vector.max(mxr[:], rel[:])
                    nc.vector.max_index(mir[:], mxr[:], rel[:])

                    oh = wpool.tile([P, 8], f32, tag="oh")
                    nc.vector.tensor_scalar(
                        oh[:], rel[:], mxr[:, :1], None, Alu.is_equal
                    )
                    cps = ps_wr.tile([1, 8], f32, tag="wr", name=f"cps{g}")
                    nc.tensor.matmul(cps[:], ones_col[:], oh[:], start=True, stop=True)
                    nc.vector.tensor_add(cnt_sb[:], cnt_sb[:], cps[:])

                    io = wpool.tile([P, 1], u32, tag="io")
                    nc.gpsimd.iota(
                        io[:], pattern=[[0, 1]], base=n0, channel_multiplier=1
                    )
                    off = wpool.tile([P, 1], u32, tag="off")
                    nc.vector.scalar_tensor_tensor(
                        out=off[:], in0=mir[:, :1], scalar=float(NT), in1=io[:],
                        op0=Alu.mult, op1=Alu.add,
                    )

                    ctx = wpool.tile([P, LAT], f32, tag="ctx")
                    nc.gpsimd.indirect_dma_start(
                        out=ctx[:], out_offset=None, in_=outs_flat,
                        in_offset=bass.IndirectOffsetOnAxis(ap=off[:], axis=0),
                    )
                    nc.scalar.dma_start(
                        ts_half[:, i, :], ts_d[n0 : n0 + P, :]
                    )
                    # transpose ts block f -> zT[:, f, i*128:...], ctx -> zT[:, 8+f, ...]
                    for f in range(LAT // P):
                        tp = ps_sm.tile([P, P], f32, tag="sm")
                        nc.tensor.transpose(
                            tp[:], ts_half[:, i, f * P : (f + 1) * P], ident[:]
                        )
                        nc.scalar.copy(zT[:, f, i * P : (i + 1) * P], tp[:])
                        nc.vector.tensor_copy(
                            zTr[:, f, i * P : (i + 1) * P],
                            zT[:, f, i * P : (i + 1) * P],
                        )
                        tp2 = ps_sm.tile([P, P], f32, tag="sm")
                        nc.tensor.transpose(
                            tp2[:], ctx[:, f * P : (f + 1) * P], ident[:]
                        )
                        nc.scalar.copy(zT[:, 8 + f, i * P : (i + 1) * P], tp2[:])
                        nc.vector.tensor_copy(
                            zTr[:, 8 + f, i * P : (i + 1) * P],
                            zT[:, 8 + f, i * P : (i + 1) * P],
                        )

                _sA.__exit__(None, None, None)
                # ---------- stage B: z_read = z @ Wread (f32r) ----------
                _sB = nc.named_scope(f"B{h}"); _sB.__enter__()
                for s in range(KO_R):       # ko sweeps of 1
                    pk = ps_wr.tile([P, HALF], f32, tag="wr", name=f"pk{h}_{s}")
                    for kb in range(KI // 4):
                        wchf = spool.tile([P, 4, P], f32, tag="wread_chf", bufs=2)
                        nc.sync.dma_start(
                            wchf[:],
                            wread_d.ap()[kb * 4 * P : (kb + 1) * 4 * P,
                                         s * P : (s + 1) * P]
                            .rearrange("(k p) n -> p k n", p=P),
                        )
                        wch = spool.tile([P, 4, P], f32r, tag="wread_ch", bufs=2)
                        nc.scalar.copy(wch[:], wchf[:])
                        for u in range(4):
                            ki = kb * 4 + u
                            nc.tensor.matmul(
                                pk[:], wch[:, u, :], zTr[:, ki, :],
                                start=(ki == 0), stop=(ki == KI - 1),
                            )
                    nc.scalar.copy(zread[:, s, :], pk[:])

                _sB.__exit__(None, None, None)
                # ---------- stage C: raw_symbol = z @ Wrs (fp32) ----------
                _sC = nc.named_scope(f"C{h}"); _sC.__enter__()
                prs = [ps_mid.tile([P, HALF], f32, tag="mid", name=f"prs{h}_{j}") for j in range(2)]
                for kb in range(KI // 2):
                    wch = spool.tile([P, 2, SYM], f32, tag="wrs_ch", bufs=2)
                    nc.sync.dma_start(
                        wch[:],
                        wrs_d.ap()[kb * 2 * P : (kb + 1) * 2 * P, :]
                        .rearrange("(k p) n -> p k n", p=P),
                    )
                    for u in range(2):
                        ki = kb * 2 + u
                        for j in range(2):
                            nc.tensor.matmul(
                                prs[j][:], wch[:, u, j * P : (j + 1) * P],
                                zT[:, ki, :],
                                start=(ki == 0), stop=(ki == KI - 1),
                            )
                for j in range(2):
                    nc.scalar.copy(rsT[:, j, :], prs[j][:])

                _sC.__exit__(None, None, None)
                # ---------- stage D: VQ argmin + quantized ----------
                _sD = nc.named_scope(f"D{h}"); _sD.__enter__()
                for i in range(TPH):
                    g = h * TPH + i
                    n0 = g * P
                    pvq = ps_wc2.tile([P, NCODE], f32, tag="wc2", name=f"pvq{g}")
                    for j in range(SYM // P):
                        nc.tensor.matmul(
                            pvq[:], rsT[:, j, i * P : (i + 1) * P], cbT[:, j, :],
                            start=(j == 0), stop=(j == SYM // P - 1),
                        )
                    sc = wpool.tile([P, NCODE], f32, tag="svq")
                    nc.vector.scalar_tensor_tensor(
                        out=sc[:], in0=pvq[:], scalar=2.0, in1=c2b[:],
                        op0=Alu.mult, op1=Alu.subtract,
                    )
                    mxv = wpool.tile([P, 8], f32, tag="mxv")
                    miv = wpool.tile([P, 8], u32, tag="miv")
                    nc.vector.max(mxv[:], sc[:])
                    nc.vector.max_index(miv[:], mxv[:], sc[:])

                    qt = wpool.tile([P, SYM], f32, tag="qt")
                    nc.gpsimd.indirect_dma_start(
                        out=qt[:], out_offset=None, in_=cb_d[:],
                        in_offset=bass.IndirectOffsetOnAxis(ap=miv[:, :1], axis=0),
                    )
                    nc.scalar.dma_start(quant_d[n0 : n0 + P, :], qt[:])
                    for j in range(SYM // P):
                        tp = ps_sm.tile([P, P], f32, tag="sm")
                        nc.tensor.transpose(
                            tp[:], qt[:, j * P : (j + 1) * P], ident[:]
                        )
                        nc.vector.tensor_copy(qT[:, j, i * P : (i + 1) * P], tp[:])
                    wide = wpool.tile([P, 32], u32, tag="wide")
                    nc.vector.memset(wide[:], 0)
                    nc.vector.tensor_copy(wide[:, :1], miv[:, :1])
                    tr = wpool.tile([P, 32], u32, tag="tr")
                    nc.vector.transpose(tr[:], wide[:])
                    nc.scalar.dma_start(
                        vqidx_d[g],
                        tr[:].rearrange("(b i) f -> b i f", i=32)[:, 0, :],
                    )

                _sD.__exit__(None, None, None)
                # ---------- stage E: h = relu([z_read, q] @ Wc1) (f32r) ----------
                _sE = nc.named_scope(f"E{h}"); _sE.__enter__()
                for s in range(KO_R):       # ko sweeps of 1
                    pc = ps_mid.tile([P, HALF], f32, tag="mid", name=f"pc{h}_{s}")
                    for kb in range(KC1 // 5):
                        wchf = spool.tile([P, 5, P], f32, tag="wc1_chf", bufs=2)
                        nc.sync.dma_start(
                            wchf[:],
                            wc1_d.ap()[kb * 5 * P : (kb + 1) * 5 * P,
                                       s * P : (s + 1) * P]
                            .rearrange("(k p) n -> p k n", p=P),
                        )
                        wch = spool.tile([P, 5, P], f32r, tag="wc1_ch", bufs=2)
                        nc.vector.tensor_copy(wch[:], wchf[:])
                        for u in range(5):
                            ki = kb * 5 + u
                            rhs = (zread[:, ki, :] if ki < KO_R
                                   else qT[:, ki - KO_R, :])
                            nc.tensor.matmul(
                                pc[:], wch[:, u, :], rhs,
                                start=(ki == 0), stop=(ki == KC1 - 1),
                            )
                    nc.scalar.activation(
                        hT[:, s, :], pc[:],
                        mybir.ActivationFunctionType.Relu,
                    )

                _sE.__exit__(None, None, None)
                # ---------- stage F: node = h @ Wc2 + ts (f32r + residual) ----------
                _sF = nc.named_scope(f"F{h}"); _sF.__enter__()
                for ko in range(2):         # output col halves of 512
                    wc2s = bpool.tile([P, KO_R, HALF], f32r, tag="wc2_ch")
                    for kb in range(KO_R // 2):
                        wchf = spool.tile([P, 2, HALF], f32, tag="wc2_chf", bufs=2)
                        nc.sync.dma_start(
                            wchf[:],
                            wc2_d.ap()[kb * 2 * P : (kb + 1) * 2 * P,
                                       ko * HALF : (ko + 1) * HALF]
                            .rearrange("(k p) n -> p k n", p=P),
                        )
                        nc.vector.tensor_copy(
                            wc2s[:, kb * 2 : (kb + 1) * 2, :], wchf[:]
                        )
                    for i in range(TPH):
                        g = h * TPH + i
                        n0 = g * P
                        pn = ps_wc2.tile([P, HALF], f32, tag="wc2")
                        for ki in range(KO_R):
                            nc.tensor.matmul(
                                pn[:], hT[:, ki, i * P : (i + 1) * P],
                                wc2s[:, ki, :],
                                start=(ki == 0), stop=(ki == KO_R - 1),
                            )
                        nod = opool.tile([P, HALF], f32, tag="nod")
                        nc.vector.tensor_add(
                            nod[:], pn[:],
                            ts_half[:, i, ko * HALF : (ko + 1) * HALF],
                        )
                        nc.scalar.dma_start(
                            node_d[n0 : n0 + P, ko * HALF : (ko + 1) * HALF],
                            nod[:],
                        )

                _sF.__exit__(None, None, None)
            nc.scalar.dma_start(tcnt_d[:], cnt_sb[:])

    nc.compile()
    return nc


def _get_nc():
    if "nc" not in _CACHE:
        _CACHE["nc"] = _build()
    return _CACHE["nc"]


def kernel(token_state, bus_symbols, bus_indices, bus_outputs, bus_mask,
           Wq, bq, Wread, bread, Wsym, bsym, Wc1, bc1, Wc2, bc2, codebook):
    from concourse.bass_utils import run_bass_kernel_spmd

    f32 = np.float32
    ts = np.ascontiguousarray(np.asarray(token_state, f32).reshape(N, LAT))
    syms = np.asarray(bus_symbols, f32).reshape(T, N, SYM)
    outs = np.asarray(bus_outputs, f32).reshape(T, N, LAT)
    Wq = np.asarray(Wq, f32)
    Wread_np = np.asarray(Wread, f32)
    Wsym_np = np.asarray(Wsym, f32)
    Wc1_np = np.asarray(Wc1, f32)
    Wc2_np = np.asarray(Wc2, f32)
    cb = np.ascontiguousarray(np.asarray(codebook, f32))

    for b in (bq, bread, bsym, bc1, bc2):
        assert not np.any(np.asarray(b)), "kernel assumes zero biases"

    wrs = np.ascontiguousarray(Wread_np @ Wsym_np)          # host-fused, fp32
    wq_row = np.ascontiguousarray(Wq.reshape(1, SYM))

    in_maps = []
    for c in range(NCORES):
        sl = slice(c * NT, (c + 1) * NT)
        in_maps.append({
            "ts": np.ascontiguousarray(ts[sl]),
            "syms": np.ascontiguousarray(syms[:, sl]),
            "outs": np.ascontiguousarray(outs[:, sl]),
            "wq": wq_row,
            "wread": np.ascontiguousarray(Wread_np),
            "wrs": wrs,
            "wc1": np.ascontiguousarray(Wc1_np),
            "wc2": np.ascontiguousarray(Wc2_np),
            "cb": cb,
        })

    nc = _get_nc()
    res = run_bass_kernel_spmd(nc, in_maps, core_ids=list(range(NCORES)))

    node = np.concatenate([res.results[c]["node"] for c in range(NCORES)], axis=0)
    quant = np.concatenate([res.results[c]["quant"] for c in range(NCORES)], axis=0)
    idx = np.concatenate(
        [res.results[c]["vqidx"].reshape(NT) for c in range(NCORES)], axis=0
    ).astype(np.int32)
    counts = np.zeros(8, np.float64)
    for c in range(NCORES):
        counts += res.results[c]["tcnt"].reshape(8).astype(np.float64)
    keep_mask = counts == 0

    return (
        node.reshape(B, S, LAT),
        quant.reshape(B, S, SYM),
        idx.reshape(B, S),
        keep_mask,
    )
